# revision 36
# baseline (speedup 1.0000x reference)
"""Trainium2 Bass kernel for nn_SSMLayer_17514876633683.

Math: the reference SSM state update broadcasts the input over H and starts
from zero state, so state[b,:,h] is identical for every h.  The layer
collapses to:
    z_t[b]    = A @ z_{t-1}[b] + B @ x[b,t]          (z in R^S, S=128)
    c[b,t]    = Cbar . z_t[b]                         (Cbar = C.mean(0))
    y_pre     = c[b,t] + (x @ D.T)[b,t,:]
    y         = LN(gelu(y_pre) + x)

Sharding: 8 cores = 4 batches x 2 time-halves, SPMD.  Half-0 cores get x
zero-padded at the front so all cores output rows 256..511 of their padded
sequence.

Device scan (Q=8 chunks, weights precomputed host-side):
  U   = B @ x^T                    over live columns         (PE, bf16)
  R_j = sum_r A^(Q-1-r) U[:, jQ+r]                           (PE, fp8)
  c^T[jj,i] = sum_L R^T G_L + sum_{k<i} U^T g-tri            (PE, fp8)
  c   -> per-row column via mask matmul; injected as the gelu bias
  xD  = x @ D^T                                              (PE, bf16)
  tail: gelu(+c) -> +x -> bn_stats LN -> out                 (Scalar/DVE)

x is fed pre-transposed from the host (no PE transpose pass).  B^T rides as
raw fp8 bytes at the front of the bf16 x^T tensor (bitcast on device; the PE
accepts fp8 lhsT with a bf16 rhs).  Scan weights are one fp8 pack; D and x
stay bf16 (fp8 there fails tolerance).  All input DMAs share the sync queue
in strict need-order — a second queue just steals HBM bandwidth from the
first-needed load.  1/sqrt(var) uses Scalar sqrt + DVE reciprocal with the
sqrt act-table load pinned right after the last gelu, where it hides under
the residual-add/bn_stats work.  Warmup/filler matmuls keep the PE clock
(HAM) from throttling between phases.
"""

import sys
from contextlib import ExitStack

sys.path.insert(0, "/opt/trn_rl_repo")

import ml_dtypes
import numpy as np

import concourse.bass as bass  # noqa: F401
import concourse.mybir as mybir
import concourse.tile as tile
from concourse import bacc, bass_utils
from concourse.tile_rust import add_dep_helper

# Problem shapes (hardcoded per the harness contract).
BSZ, T, H, S = 4, 512, 512, 128
Q = 8            # scan chunk length
NCH = T // Q     # 64 chunks
TOUT = 256       # output rows per core
LN_EPS = 1e-5
NCORES = 8
NWARM = 10       # upfront PE warmup matmuls
TRUNC_TOL = 1e-5

F32 = mybir.dt.float32
F16 = mybir.dt.float16
BF16 = mybir.dt.bfloat16
FP8 = mybir.dt.float8e4
BF16_NP = ml_dtypes.bfloat16
FP8_NP = ml_dtypes.float8_e4m3fn
AF = mybir.ActivationFunctionType
ALU = mybir.AluOpType


def _host_weights(A, Bm, Cm):
    """Precompute scan weights; returns (APOW, APQL, WTRI, LZ) float64."""
    A64 = A.astype(np.float64)
    Cbar = Cm.astype(np.float64).mean(axis=0)          # (S,)

    pows = [np.eye(S)]
    for _ in range(Q):
        pows.append(pows[-1] @ A64)                    # pows[k] = A^k
    AQm = pows[Q]

    q8 = [np.eye(S)]
    while len(q8) < NCH - 1:
        nxt = q8[-1] @ AQm
        if np.linalg.norm(nxt, 2) < TRUNC_TOL:
            break
        q8.append(nxt)
    LZ = len(q8)

    g = [pows[k].T @ Cbar for k in range(Q)]           # g_k = (A^T)^k Cbar
    G8 = np.stack(g, axis=1)                           # (S, Q)
    APOW = np.concatenate([pows[Q - 1 - r].T for r in range(Q)], axis=1)
    APQL = np.concatenate([m.T @ G8 for m in q8], axis=1)    # (S, LZ*Q)
    WTRI = np.zeros((S, Q * Q))
    for k in range(Q):
        for i in range(Q):
            if i > k:
                WTRI[:, k * Q + i] = g[i - 1 - k]
    return APOW, APQL, WTRI, LZ


def _emit(tc, aps, LZ):
    nc = tc.nc
    xtb, pw, pd, xr, yout = (aps["xtb"], aps["pw"], aps["pd"], aps["xr"],
                             aps["yout"])
    ju0 = NCH // 2 - LZ
    nchr = NCH - ju0
    NU = nchr * Q
    offD = NU - 256          # xt column where t=256 starts
    XW = 256 + 4 * NU        # xtb: fp8 B^T pack (256 bf16 slots) | 4 xt blocks
    o_ql = Q * S             # fp8 pack offsets
    o_wt = o_ql + LZ * Q
    o_mk = o_wt + Q * Q

    ctx = ExitStack()
    cpool = ctx.enter_context(tc.tile_pool(name="const", bufs=1))
    wpool = ctx.enter_context(tc.tile_pool(name="work", bufs=1))
    psp = ctx.enter_context(tc.tile_pool(name="psp", bufs=1, space="PSUM"))

    # ---- input DMAs first, all on one queue in strict need-order ----------
    # (a second queue would steal HBM bandwidth from the first-needed load)
    XTB = cpool.tile([128, XW], BF16, tag="XTB")
    nc.sync.dma_start(XTB[:], xtb)
    BT8 = XTB[:, 0:256].bitcast(FP8)

    def xt_blk(hh, c0, c1):
        return XTB[:, 256 + hh * NU + c0:256 + hh * NU + c1]
    PW = cpool.tile([128, o_mk + 256], FP8, tag="PW")
    nc.sync.dma_start(PW[:], pw)
    PD = cpool.tile([128, 4, H], BF16, tag="PD")
    nc.sync.dma_start(PD[:], pd.rearrange("p (hh o) -> p hh o", hh=4))
    XR = cpool.tile([128, 2, H], BF16, tag="XR")
    nc.sync.dma_start(XR[:], xr.rearrange("p (tt h) -> p tt h", tt=2))

    # ---- PE warmup (one accumulation group trips the HAM un-throttle) -----
    warm_sb = cpool.tile([128, 256], BF16, tag="warm_sb")
    nc.vector.memset(warm_sb[:], 0.0)
    wp = psp.tile([128, 256], F32, tag="wp", name="warm_ps")
    for i in range(NWARM):
        nc.tensor.matmul(wp[:], lhsT=warm_sb[:, :128], rhs=warm_sb[:],
                         start=(i == 0), stop=(i == NWARM - 1))

    def fillers(n, after=None):
        for _ in range(n):
            mi = nc.tensor.matmul(wp[:], lhsT=warm_sb[:, :128], rhs=warm_sb[:],
                                  start=True, stop=True)
            if after is not None:
                add_dep_helper(after, mi.ins, False, "pin filler")

    # ---- scalar: gelu act-table preload while DMAs stream -----------------
    gsc = cpool.tile([128, 1], F32, tag="gsc")
    nc.vector.memset(gsc[:], 0.0)
    nc.scalar.activation(gsc[:], gsc[:], AF.Gelu)

    ones_sb = cpool.tile([32, 1], BF16, tag="ones_sb")
    nc.vector.memset(ones_sb[:], 1.0)
    eps_sb = cpool.tile([128, 1], F32, tag="eps_sb")
    nc.vector.memset(eps_sb[:], LN_EPS)
    big_sb = cpool.tile([128, 1], F32, tag="big_sb")
    nc.vector.memset(big_sb[:], 1.0e4)

    # ---- U = B @ x^T over live columns (S x NU) ---------------------------
    U_ps = psp.tile([128, NU], F32, tag="U_ps")
    for hh in range(4):
        last_u = nc.tensor.matmul(U_ps[:], lhsT=BT8[:, hh * S:(hh + 1) * S],
                                  rhs=xt_blk(hh, 0, NU),
                                  start=(hh == 0), stop=(hh == 3))
    U_sb = cpool.tile([128, NU], FP8, tag="U_sb")
    U_sb3 = U_sb.rearrange("s (r j) -> s r j", r=Q)
    U_ps3 = U_ps.rearrange("s (j r) -> s r j", j=nchr)
    nc.vector.tensor_copy(U_sb3[:, 0:Q // 2, :], U_ps3[:, 0:Q // 2, :])
    nc.vector.tensor_copy(U_sb3[:, Q // 2:Q, :], U_ps3[:, Q // 2:Q, :])
    fillers(4, after=last_u.ins)

    # ---- chunk summaries R ------------------------------------------------
    R_ps = psp.tile([128, nchr], F32, tag="R_ps")
    for r in range(Q):
        last_r = nc.tensor.matmul(R_ps[:], lhsT=PW[:, r * S:(r + 1) * S],
                                  rhs=U_sb3[:, r, :], start=(r == 0),
                                  stop=(r == Q - 1))
    R_sb = cpool.tile([128, nchr], FP8, tag="R_sb")
    nc.vector.tensor_copy(R_sb[:], R_ps[:])
    fillers(2, after=last_r.ins)

    # ---- c^T for the output half (jj in [0,32)) ---------------------------
    c_ps = psp.tile([32, Q], F32, tag="c_ps")
    for L in range(LZ):
        base = LZ - 1 - L
        nc.tensor.matmul(c_ps[:], lhsT=R_sb[:, base:base + NCH // 2],
                         rhs=PW[:, o_ql + L * Q:o_ql + (L + 1) * Q],
                         start=(L == 0), stop=False)
    for k in range(Q):
        last_c = nc.tensor.matmul(
            c_ps[:], lhsT=U_sb3[:, k, LZ:LZ + NCH // 2],
            rhs=PW[:, o_wt + k * Q:o_wt + (k + 1) * Q],
            start=False, stop=(k == Q - 1))

    # ---- scatter c into per-row columns, reduce to c_col ------------------
    # lhsTc[j, m*8+i] = c^T[j, i] * [j == m];  ccol[p] = sum_j lhsTc[j, p]
    lhsTc = cpool.tile([32, 256], BF16, tag="lhsTc")
    c_bc = c_ps[:, None, :].to_broadcast((32, 32, Q))
    nc.vector.tensor_tensor(
        lhsTc.rearrange("j (m i) -> j m i", m=32), c_bc,
        PW[0:32, o_mk:o_mk + 256].rearrange("j (m i) -> j m i", m=32),
        ALU.mult)
    ccol_ps = psp.tile([128, 2], F32, tag="ccol_ps")
    for n in range(2):
        nc.tensor.matmul(ccol_ps[:, n:n + 1],
                         lhsT=lhsTc[:, n * 128:(n + 1) * 128],
                         rhs=ones_sb[:], start=True, stop=True)
    ccol_sb = wpool.tile([128, 2], F32, tag="ccol_sb")
    nc.vector.tensor_copy(ccol_sb[:, 0:1], ccol_ps[:, 0:1])
    nc.vector.tensor_copy(ccol_sb[:, 1:2], ccol_ps[:, 1:2])
    fillers(2, after=last_c.ins)

    # ---- xD (pinned after the scan; half-width PSUM groups so each tile's
    # gelu can start after half the xD work) ---------------------------------
    y_pss = []
    prev_last = last_c
    for tt2 in range(2):
        y_ps = psp.tile([128, H], F32, tag=f"y_ps{tt2}", name=f"y_ps{tt2}")
        c0 = offD + tt2 * 128
        for half in range(2):
            hs = slice(half * (H // 2), (half + 1) * (H // 2))
            for hh in range(4):
                mm = nc.tensor.matmul(y_ps[:, hs],
                                      lhsT=xt_blk(hh, c0, c0 + 128),
                                      rhs=PD[:, hh, hs], start=(hh == 0),
                                      stop=(hh == 3))
                if hh == 0:
                    add_dep_helper(mm.ins, prev_last.ins, False,
                                   "keep PE order")
            prev_last = mm
        y_pss.append(y_ps)

    # ---- tail: gelu(+c) -> +x -> LN stats -> normalize -> out -------------
    # per-half gelu/add pipelines PSUM->Scalar->(DVE|GpSimd) per 256 cols
    y_sbs, mvs, aggrs = [], [], []
    for tt2 in range(2):
        g_sb = wpool.tile([128, H], F32, tag=f"g_sb{tt2}", name=f"g_sb{tt2}")
        y_sb = wpool.tile([128, H], F32, tag=f"y_sb{tt2}", name=f"y_sb{tt2}")
        add_eng = nc.vector if tt2 == 0 else nc.gpsimd
        hb = ccol_sb[:, tt2:tt2 + 1]
        for half in range(2):
            hs = slice(half * (H // 2), (half + 1) * (H // 2))
            gel_i = nc.scalar.activation(g_sb[:, hs], y_pss[tt2][:, hs],
                                         AF.Gelu, bias=hb, scale=1.0)
            add_eng.tensor_add(y_sb[:, hs], g_sb[:, hs], XR[:, tt2, hs])
        if tt2 == 1:
            # sqrt act-table load starts right after the last gelu; it hides
            # under the residual add + bn_stats work on the other engines
            sq_scr = wpool.tile([128, 1], F32, tag="sq_scr")
            dm_i = nc.scalar.activation(sq_scr[:], ccol_sb[:, 0:1], AF.Sqrt,
                                        bias=big_sb[:], scale=1.0)
            add_dep_helper(dm_i.ins, gel_i.ins, False, "table after gelus")
        st6 = wpool.tile([128, 6], F32, tag=f"st6_{tt2}", name=f"st6_{tt2}")
        st_i = nc.vector.bn_stats(st6[:], y_sb[:])
        if tt2 == 1:
            # tile0's aggregate must not queue behind tile1's stats
            add_dep_helper(st_i.ins, aggrs[0].ins, False, "aggr0 first")
        mv = wpool.tile([128, 2], F32, tag=f"mv{tt2}", name=f"mv{tt2}")
        ag_i = nc.vector.bn_aggr(mv[:], st6[:])
        y_sbs.append(y_sb)
        mvs.append(mv)
        aggrs.append(ag_i)

    for tt2 in range(2):
        sd = wpool.tile([128, 1], F32, tag=f"sd{tt2}", name=f"sd{tt2}")
        nc.scalar.activation(sd[:], mvs[tt2][:, 1:2], AF.Sqrt, bias=eps_sb[:],
                             scale=1.0)
        iv = wpool.tile([128, 1], F32, tag=f"iv{tt2}", name=f"iv{tt2}")
        nc.vector.reciprocal(iv[:], sd[:])
        o_sb = wpool.tile([128, H], F16, tag=f"o_sb{tt2}", name=f"o_sb{tt2}")
        nc.vector.tensor_scalar(o_sb[:], y_sbs[tt2][:], mvs[tt2][:, 0:1],
                                iv[:], op0=ALU.subtract, op1=ALU.mult)
        # tile0's store rides the idle scalar queue so tile1's (the last
        # instruction that gates the epilogue) never queues behind it
        dma_eng = nc.scalar if tt2 == 0 else nc.sync
        dma_eng.dma_start(yout[tt2 * 128:(tt2 + 1) * 128, :], o_sb[:])

    ctx.close()


def _build_program(LZ):
    nc = bacc.Bacc("TRN2", target_bir_lowering=False, debug=False,
                   enable_asserts=False, num_devices=NCORES)
    ju0 = NCH // 2 - LZ
    NU = (NCH - ju0) * Q
    CW = 128 + NU
    NW = Q * S + LZ * Q + Q * Q + 256
    aps = {
        "xtb": nc.dram_tensor("xtb", (128, 256 + 4 * NU), BF16,
                              kind="ExternalInput").ap(),
        "pw": nc.dram_tensor("pw", (128, NW), FP8, kind="ExternalInput").ap(),
        "pd": nc.dram_tensor("pd", (128, 4 * H), BF16,
                             kind="ExternalInput").ap(),
        "xr": nc.dram_tensor("xr", (128, 2 * H), BF16,
                             kind="ExternalInput").ap(),
        "yout": nc.dram_tensor("yout", (TOUT, H), F16,
                               kind="ExternalOutput").ap(),
    }
    with tile.TileContext(nc) as tc:
        _emit(tc, aps, LZ)
    nc.compile()
    return nc


def _prepare_in_maps(x, A, Bm, Cm, D):
    APOW, APQL, WTRI, LZ = _host_weights(A, Bm, Cm)
    ju0 = NCH // 2 - LZ
    NU = (NCH - ju0) * Q

    msk = np.zeros((128, 256))
    for j in range(32):
        msk[j, j * Q:(j + 1) * Q] = 1.0
    pw = np.concatenate([APOW, APQL, WTRI, msk], axis=1)
    pw8 = np.clip(pw, -240, 240).astype(FP8_NP)

    # pd[p, hh*H+o] = D[o, hh*128+p]
    pd = np.ascontiguousarray(
        D.T.reshape(4, 128, H).transpose(1, 0, 2).reshape(128, 4 * H)
    ).astype(BF16_NP)

    Bt = Bm.T  # (H, S)
    in_maps = []
    for core in range(NCORES):
        b, half = core // 2, core % 2
        if half == 0:
            xb = np.concatenate(
                [np.zeros((TOUT, H), np.float32), x[b, :TOUT]], axis=0)
        else:
            xb = x[b]
        xtf = xb.T  # (H, T)
        bt8 = np.empty((128, 4 * S), FP8_NP)
        for hh in range(4):
            bt8[:, hh * S:(hh + 1) * S] = np.clip(
                Bt[hh * 128:(hh + 1) * 128, :], -240, 240).astype(FP8_NP)
        btv = bt8.view(np.uint16).view(BF16_NP)    # (128, 256) raw fp8 pairs
        blocks = [btv] + [
            np.ascontiguousarray(xtf[hh * 128:(hh + 1) * 128, ju0 * Q:]
                                 ).astype(BF16_NP)
            for hh in range(4)]
        xtb = np.concatenate(blocks, axis=1)       # (128, 256 + 4*NU)
        xr = np.ascontiguousarray(
            xb[256:].reshape(2, 128, H).transpose(1, 0, 2).reshape(128, 2 * H))
        in_maps.append({
            "xtb": np.ascontiguousarray(xtb),
            "pw": pw8,
            "pd": pd,
            "xr": xr.astype(BF16_NP),
        })
    return in_maps, LZ


def _run(inputs, trace=False):
    x = np.asarray(inputs["x"], np.float32)
    A = np.asarray(inputs["A"], np.float32)
    Bm = np.asarray(inputs["B"], np.float32)
    Cm = np.asarray(inputs["C"], np.float32)
    D = np.asarray(inputs["D"], np.float32)
    gamma = np.asarray(inputs["gamma"], np.float32)
    beta = np.asarray(inputs["beta"], np.float32)

    in_maps, LZ = _prepare_in_maps(x, A, Bm, Cm, D)
    nc = _build_program(LZ)
    res = bass_utils.run_bass_kernel_spmd(
        nc, in_maps, core_ids=list(range(NCORES)), trace=trace)
    y = np.empty((BSZ, T, H), np.float32)
    for core in range(NCORES):
        b, half = core // 2, core % 2
        y[b, half * TOUT:(half + 1) * TOUT, :] = (
            res.results[core]["yout"].astype(np.float32))
    # gamma/beta are ones/zeros in this problem; apply anyway for safety
    if not (np.all(gamma == 1.0) and np.all(beta == 0.0)):
        y = y * gamma + beta
    return y, res


def kernel(**inputs):
    y, _ = _run(inputs, trace=False)
    return y


def kernel_traced(**inputs):
    return _run(inputs, trace=True)


# revision 37
# speedup vs baseline: 1.0689x; 1.0689x over previous
"""Trainium2 Bass kernel for nn_SSMLayer_17514876633683.

Math: the reference SSM state update broadcasts the input over H and starts
from zero state, so state[b,:,h] is identical for every h.  The layer
collapses to:
    z_t[b]    = A @ z_{t-1}[b] + B @ x[b,t]          (z in R^S, S=128)
    c[b,t]    = Cbar . z_t[b]                         (Cbar = C.mean(0))
    y_pre     = c[b,t] + (x @ D.T)[b,t,:]
    y         = LN(gelu(y_pre) + x)

Sharding: 8 cores = 4 batches x 2 time-halves, SPMD.  Half-0 cores get x
zero-padded at the front so all cores output rows 256..511 of their padded
sequence.

Device scan (Q=8 chunks, weights precomputed host-side):
  U   = B @ x^T                    over live columns         (PE, bf16)
  R_j = sum_r A^(Q-1-r) U[:, jQ+r]                           (PE, fp8)
  c^T[jj,i] = sum_L R^T G_L + sum_{k<i} U^T g-tri            (PE, fp8)
  c   -> per-row column via mask matmul; injected as the gelu bias
  xD  = x @ D^T                                              (PE, bf16)
  tail: gelu(+c) -> +x -> bn_stats LN -> out                 (Scalar/DVE)

x is fed pre-transposed from the host (no PE transpose pass).  B^T rides as
raw fp8 bytes at the front of the bf16 x^T tensor (bitcast on device; the PE
accepts fp8 lhsT with a bf16 rhs).  Scan weights are one fp8 pack; D and x
stay bf16 (fp8 there fails tolerance).  All input DMAs share the sync queue
in strict need-order — a second queue just steals HBM bandwidth from the
first-needed load.  1/sqrt(var) uses Scalar sqrt + DVE reciprocal with the
sqrt act-table load pinned right after the last gelu, where it hides under
the residual-add/bn_stats work.  Warmup/filler matmuls keep the PE clock
(HAM) from throttling between phases.
"""

import sys
from contextlib import ExitStack

sys.path.insert(0, "/opt/trn_rl_repo")

import ml_dtypes
import numpy as np

import concourse.bass as bass  # noqa: F401
import concourse.mybir as mybir
import concourse.tile as tile
from concourse import bacc, bass_utils
from concourse.tile_rust import add_dep_helper

# Problem shapes (hardcoded per the harness contract).
BSZ, T, H, S = 4, 512, 512, 128
Q = 8            # scan chunk length
NCH = T // Q     # 64 chunks
TOUT = 256       # output rows per core
LN_EPS = 1e-5
NCORES = 8
NWARM = 10       # upfront PE warmup matmuls
TRUNC_TOL = 1e-5

F32 = mybir.dt.float32
F16 = mybir.dt.float16
BF16 = mybir.dt.bfloat16
FP8 = mybir.dt.float8e4
BF16_NP = ml_dtypes.bfloat16
FP8_NP = ml_dtypes.float8_e4m3fn
AF = mybir.ActivationFunctionType
ALU = mybir.AluOpType


def _host_weights(A, Bm, Cm):
    """Precompute scan weights; returns (APOW, APQL, WTRI, LZ) float64."""
    A64 = A.astype(np.float64)
    Cbar = Cm.astype(np.float64).mean(axis=0)          # (S,)

    pows = [np.eye(S)]
    for _ in range(Q):
        pows.append(pows[-1] @ A64)                    # pows[k] = A^k
    AQm = pows[Q]

    q8 = [np.eye(S)]
    while len(q8) < NCH - 1:
        nxt = q8[-1] @ AQm
        if np.linalg.norm(nxt, 2) < TRUNC_TOL:
            break
        q8.append(nxt)
    LZ = len(q8)

    g = [pows[k].T @ Cbar for k in range(Q)]           # g_k = (A^T)^k Cbar
    G8 = np.stack(g, axis=1)                           # (S, Q)
    APOW = np.concatenate([pows[Q - 1 - r].T for r in range(Q)], axis=1)
    APQL = np.concatenate([m.T @ G8 for m in q8], axis=1)    # (S, LZ*Q)
    WTRI = np.zeros((S, Q * Q))
    for k in range(Q):
        for i in range(Q):
            if i > k:
                WTRI[:, k * Q + i] = g[i - 1 - k]
    return APOW, APQL, WTRI, LZ


def _emit(tc, aps, LZ):
    nc = tc.nc
    xtb, pw, pd, xr, yout = (aps["xtb"], aps["pw"], aps["pd"], aps["xr"],
                             aps["yout"])
    ju0 = NCH // 2 - LZ
    nchr = NCH - ju0
    NU = nchr * Q
    offD = NU - 256          # xt column where t=256 starts
    XW = 256 + 4 * NU        # xtb: fp8 B^T pack (256 bf16 slots) | 4 xt blocks
    o_ql = Q * S             # fp8 pack offsets
    o_wt = o_ql + LZ * Q
    o_mk = o_wt + Q * Q

    ctx = ExitStack()
    cpool = ctx.enter_context(tc.tile_pool(name="const", bufs=1))
    wpool = ctx.enter_context(tc.tile_pool(name="work", bufs=1))
    psp = ctx.enter_context(tc.tile_pool(name="psp", bufs=1, space="PSUM"))

    # ---- input DMAs first, all on one queue in strict need-order ----------
    # (a second queue would steal HBM bandwidth from the first-needed load)
    XTB = cpool.tile([128, XW], BF16, tag="XTB")
    nc.sync.dma_start(XTB[:], xtb)
    BT8 = XTB[:, 0:256].bitcast(FP8)

    def xt_blk(hh, c0, c1):
        return XTB[:, 256 + hh * NU + c0:256 + hh * NU + c1]
    PW = cpool.tile([128, o_mk + 256], FP8, tag="PW")
    nc.sync.dma_start(PW[:], pw)
    PD = cpool.tile([128, 4, H], BF16, tag="PD")
    nc.sync.dma_start(PD[:], pd.rearrange("p (hh o) -> p hh o", hh=4))
    XR = cpool.tile([128, 2, H], BF16, tag="XR")
    nc.sync.dma_start(XR[:], xr.rearrange("p (tt h) -> p tt h", tt=2))

    # ---- PE warmup (one accumulation group trips the HAM un-throttle) -----
    warm_sb = cpool.tile([128, 256], BF16, tag="warm_sb")
    nc.vector.memset(warm_sb[:], 0.0)
    wp = psp.tile([128, 256], F32, tag="wp", name="warm_ps")
    for i in range(NWARM):
        nc.tensor.matmul(wp[:], lhsT=warm_sb[:, :128], rhs=warm_sb[:],
                         start=(i == 0), stop=(i == NWARM - 1))

    def fillers(n, after=None):
        for _ in range(n):
            mi = nc.tensor.matmul(wp[:], lhsT=warm_sb[:, :128], rhs=warm_sb[:],
                                  start=True, stop=True)
            if after is not None:
                add_dep_helper(after, mi.ins, False, "pin filler")

    # ---- scalar: gelu act-table preload while DMAs stream -----------------
    gsc = cpool.tile([128, 1], F32, tag="gsc")
    nc.vector.memset(gsc[:], 0.0)
    nc.scalar.activation(gsc[:], gsc[:], AF.Gelu)

    ones_sb = cpool.tile([32, 1], BF16, tag="ones_sb")
    nc.vector.memset(ones_sb[:], 1.0)
    eps_sb = cpool.tile([128, 1], F32, tag="eps_sb")
    nc.vector.memset(eps_sb[:], LN_EPS)
    big_sb = cpool.tile([128, 1], F32, tag="big_sb")
    nc.vector.memset(big_sb[:], 1.0e4)

    # ---- U = B @ x^T over live columns (S x NU) ---------------------------
    U_ps = psp.tile([128, NU], F32, tag="U_ps")
    for hh in range(4):
        last_u = nc.tensor.matmul(U_ps[:], lhsT=BT8[:, hh * S:(hh + 1) * S],
                                  rhs=xt_blk(hh, 0, NU),
                                  start=(hh == 0), stop=(hh == 3))
    U_sb = cpool.tile([128, NU], FP8, tag="U_sb")
    U_sb3 = U_sb.rearrange("s (r j) -> s r j", r=Q)
    U_ps3 = U_ps.rearrange("s (j r) -> s r j", j=nchr)
    nc.vector.tensor_copy(U_sb3[:, 0:Q // 2, :], U_ps3[:, 0:Q // 2, :])
    nc.vector.tensor_copy(U_sb3[:, Q // 2:Q, :], U_ps3[:, Q // 2:Q, :])
    fillers(4, after=last_u.ins)

    # ---- chunk summaries R ------------------------------------------------
    R_ps = psp.tile([128, nchr], F32, tag="R_ps")
    for r in range(Q):
        last_r = nc.tensor.matmul(R_ps[:], lhsT=PW[:, r * S:(r + 1) * S],
                                  rhs=U_sb3[:, r, :], start=(r == 0),
                                  stop=(r == Q - 1))
    R_sb = cpool.tile([128, nchr], FP8, tag="R_sb")
    nc.vector.tensor_copy(R_sb[:], R_ps[:])
    fillers(2, after=last_r.ins)

    # ---- c^T for the output half (jj in [0,32)) ---------------------------
    c_ps = psp.tile([32, Q], F32, tag="c_ps")
    for L in range(LZ):
        base = LZ - 1 - L
        nc.tensor.matmul(c_ps[:], lhsT=R_sb[:, base:base + NCH // 2],
                         rhs=PW[:, o_ql + L * Q:o_ql + (L + 1) * Q],
                         start=(L == 0), stop=False)
    for k in range(Q):
        last_c = nc.tensor.matmul(
            c_ps[:], lhsT=U_sb3[:, k, LZ:LZ + NCH // 2],
            rhs=PW[:, o_wt + k * Q:o_wt + (k + 1) * Q],
            start=False, stop=(k == Q - 1))

    # ---- scatter c into per-row columns, reduce to c_col ------------------
    # lhsTc[j, m*8+i] = c^T[j, i] * [j == m];  ccol[p] = sum_j lhsTc[j, p]
    lhsTc = cpool.tile([32, 256], BF16, tag="lhsTc")
    c_bc = c_ps[:, None, :].to_broadcast((32, 32, Q))
    nc.vector.tensor_tensor(
        lhsTc.rearrange("j (m i) -> j m i", m=32), c_bc,
        PW[0:32, o_mk:o_mk + 256].rearrange("j (m i) -> j m i", m=32),
        ALU.mult)
    ccol_ps = psp.tile([128, 2], F32, tag="ccol_ps")
    for n in range(2):
        nc.tensor.matmul(ccol_ps[:, n:n + 1],
                         lhsT=lhsTc[:, n * 128:(n + 1) * 128],
                         rhs=ones_sb[:], start=True, stop=True)
    ccol_sb = wpool.tile([128, 2], F32, tag="ccol_sb")
    nc.vector.tensor_copy(ccol_sb[:, 0:1], ccol_ps[:, 0:1])
    nc.vector.tensor_copy(ccol_sb[:, 1:2], ccol_ps[:, 1:2])
    fillers(2, after=last_c.ins)

    # ---- xD (pinned after the scan so the PE queue stays in chain order) --
    y_pss = []
    prev_last = last_c
    for tt2 in range(2):
        y_ps = psp.tile([128, H], F32, tag=f"y_ps{tt2}", name=f"y_ps{tt2}")
        c0 = offD + tt2 * 128
        for hh in range(4):
            mm = nc.tensor.matmul(y_ps[:], lhsT=xt_blk(hh, c0, c0 + 128),
                                  rhs=PD[:, hh, :], start=(hh == 0),
                                  stop=(hh == 3))
            if hh == 0:
                add_dep_helper(mm.ins, prev_last.ins, False, "keep PE order")
        prev_last = mm
        y_pss.append(y_ps)

    # ---- tail: gelu(+c) -> +x -> LN stats -> normalize -> out -------------
    y_sbs, mvs, aggrs = [], [], []
    for tt2 in range(2):
        g_sb = wpool.tile([128, H], F32, tag=f"g_sb{tt2}", name=f"g_sb{tt2}")
        y_sb = wpool.tile([128, H], F32, tag=f"y_sb{tt2}", name=f"y_sb{tt2}")
        if tt2 == 0:
            nc.scalar.activation(g_sb[:], y_pss[tt2][:], AF.Gelu,
                                 bias=ccol_sb[:, tt2:tt2 + 1], scale=1.0)
            nc.vector.tensor_add(y_sb[:], g_sb[:], XR[:, tt2, :])
        else:
            # halves pipeline Scalar->GpSimd so the slow GpSimd add starts
            # half a gelu earlier on this (critical) tile
            hb = ccol_sb[:, 1:2]
            nc.scalar.activation(g_sb[:, 0:H // 2], y_pss[1][:, 0:H // 2],
                                 AF.Gelu, bias=hb, scale=1.0)
            gel_i = nc.scalar.activation(g_sb[:, H // 2:H],
                                         y_pss[1][:, H // 2:H],
                                         AF.Gelu, bias=hb, scale=1.0)
            # sqrt act-table load starts right after the last gelu; it hides
            # under the residual add + bn_stats work on the other engines
            sq_scr = wpool.tile([128, 1], F32, tag="sq_scr")
            dm_i = nc.scalar.activation(sq_scr[:], ccol_sb[:, 0:1], AF.Sqrt,
                                        bias=big_sb[:], scale=1.0)
            add_dep_helper(dm_i.ins, gel_i.ins, False, "table after gelus")
            nc.gpsimd.tensor_add(y_sb[:, 0:H // 2], g_sb[:, 0:H // 2],
                                 XR[:, 1, 0:H // 2])
            nc.gpsimd.tensor_add(y_sb[:, H // 2:H], g_sb[:, H // 2:H],
                                 XR[:, 1, H // 2:H])
        st6 = wpool.tile([128, 6], F32, tag=f"st6_{tt2}", name=f"st6_{tt2}")
        st_i = nc.vector.bn_stats(st6[:], y_sb[:])
        if tt2 == 1:
            # tile0's aggregate must not queue behind tile1's stats
            add_dep_helper(st_i.ins, aggrs[0].ins, False, "aggr0 first")
        mv = wpool.tile([128, 2], F32, tag=f"mv{tt2}", name=f"mv{tt2}")
        ag_i = nc.vector.bn_aggr(mv[:], st6[:])
        y_sbs.append(y_sb)
        mvs.append(mv)
        aggrs.append(ag_i)

    for tt2 in range(2):
        sd = wpool.tile([128, 1], F32, tag=f"sd{tt2}", name=f"sd{tt2}")
        nc.scalar.activation(sd[:], mvs[tt2][:, 1:2], AF.Sqrt, bias=eps_sb[:],
                             scale=1.0)
        iv = wpool.tile([128, 1], F32, tag=f"iv{tt2}", name=f"iv{tt2}")
        nc.vector.reciprocal(iv[:], sd[:])
        o_sb = wpool.tile([128, H], F16, tag=f"o_sb{tt2}", name=f"o_sb{tt2}")
        nc.vector.tensor_scalar(o_sb[:], y_sbs[tt2][:], mvs[tt2][:, 0:1],
                                iv[:], op0=ALU.subtract, op1=ALU.mult)
        # tile0's store rides the idle scalar queue so tile1's (the last
        # instruction that gates the epilogue) never queues behind it
        dma_eng = nc.scalar if tt2 == 0 else nc.sync
        dma_eng.dma_start(yout[tt2 * 128:(tt2 + 1) * 128, :], o_sb[:])

    ctx.close()


def _build_program(LZ):
    nc = bacc.Bacc("TRN2", target_bir_lowering=False, debug=False,
                   enable_asserts=False, num_devices=NCORES)
    ju0 = NCH // 2 - LZ
    NU = (NCH - ju0) * Q
    CW = 128 + NU
    NW = Q * S + LZ * Q + Q * Q + 256
    aps = {
        "xtb": nc.dram_tensor("xtb", (128, 256 + 4 * NU), BF16,
                              kind="ExternalInput").ap(),
        "pw": nc.dram_tensor("pw", (128, NW), FP8, kind="ExternalInput").ap(),
        "pd": nc.dram_tensor("pd", (128, 4 * H), BF16,
                             kind="ExternalInput").ap(),
        "xr": nc.dram_tensor("xr", (128, 2 * H), BF16,
                             kind="ExternalInput").ap(),
        "yout": nc.dram_tensor("yout", (TOUT, H), F16,
                               kind="ExternalOutput").ap(),
    }
    with tile.TileContext(nc) as tc:
        _emit(tc, aps, LZ)
    nc.compile()
    return nc


def _prepare_in_maps(x, A, Bm, Cm, D):
    APOW, APQL, WTRI, LZ = _host_weights(A, Bm, Cm)
    ju0 = NCH // 2 - LZ
    NU = (NCH - ju0) * Q

    msk = np.zeros((128, 256))
    for j in range(32):
        msk[j, j * Q:(j + 1) * Q] = 1.0
    pw = np.concatenate([APOW, APQL, WTRI, msk], axis=1)
    pw8 = np.clip(pw, -240, 240).astype(FP8_NP)

    # pd[p, hh*H+o] = D[o, hh*128+p]
    pd = np.ascontiguousarray(
        D.T.reshape(4, 128, H).transpose(1, 0, 2).reshape(128, 4 * H)
    ).astype(BF16_NP)

    Bt = Bm.T  # (H, S)
    in_maps = []
    for core in range(NCORES):
        b, half = core // 2, core % 2
        if half == 0:
            xb = np.concatenate(
                [np.zeros((TOUT, H), np.float32), x[b, :TOUT]], axis=0)
        else:
            xb = x[b]
        xtf = xb.T  # (H, T)
        bt8 = np.empty((128, 4 * S), FP8_NP)
        for hh in range(4):
            bt8[:, hh * S:(hh + 1) * S] = np.clip(
                Bt[hh * 128:(hh + 1) * 128, :], -240, 240).astype(FP8_NP)
        btv = bt8.view(np.uint16).view(BF16_NP)    # (128, 256) raw fp8 pairs
        blocks = [btv] + [
            np.ascontiguousarray(xtf[hh * 128:(hh + 1) * 128, ju0 * Q:]
                                 ).astype(BF16_NP)
            for hh in range(4)]
        xtb = np.concatenate(blocks, axis=1)       # (128, 256 + 4*NU)
        xr = np.ascontiguousarray(
            xb[256:].reshape(2, 128, H).transpose(1, 0, 2).reshape(128, 2 * H))
        in_maps.append({
            "xtb": np.ascontiguousarray(xtb),
            "pw": pw8,
            "pd": pd,
            "xr": xr.astype(BF16_NP),
        })
    return in_maps, LZ


def _run(inputs, trace=False):
    x = np.asarray(inputs["x"], np.float32)
    A = np.asarray(inputs["A"], np.float32)
    Bm = np.asarray(inputs["B"], np.float32)
    Cm = np.asarray(inputs["C"], np.float32)
    D = np.asarray(inputs["D"], np.float32)
    gamma = np.asarray(inputs["gamma"], np.float32)
    beta = np.asarray(inputs["beta"], np.float32)

    in_maps, LZ = _prepare_in_maps(x, A, Bm, Cm, D)
    nc = _build_program(LZ)
    res = bass_utils.run_bass_kernel_spmd(
        nc, in_maps, core_ids=list(range(NCORES)), trace=trace)
    y = np.empty((BSZ, T, H), np.float32)
    for core in range(NCORES):
        b, half = core // 2, core % 2
        y[b, half * TOUT:(half + 1) * TOUT, :] = (
            res.results[core]["yout"].astype(np.float32))
    # gamma/beta are ones/zeros in this problem; apply anyway for safety
    if not (np.all(gamma == 1.0) and np.all(beta == 0.0)):
        y = y * gamma + beta
    return y, res


def kernel(**inputs):
    y, _ = _run(inputs, trace=False)
    return y


def kernel_traced(**inputs):
    return _run(inputs, trace=True)
